# revision 20
# baseline (speedup 1.0000x reference)
"""Trainium2 Bass kernel for DenseIouPred.

The reference op only consumes output[0,0] (4,W,H), target[0,0] (4,),
ind[0,0,0] (scalar) and emits a (W,H) f32 IoU map that is nonzero only
inside a +/-radius window around the center decoded from `ind`.

Every validity condition of the reference (shifted target box
nonnegative, offset within radius, center+offset inside the image) is an
interval constraint in the row/column offsets, so the nonzero output is
exactly a rectangle [v_lo..v_hi] x [u_lo..u_hi] around the center. The
host does the index bookkeeping that involves no tensor math — decodes
(ch, cw) from the scalar `ind`, derives the rectangle, slices the
(4, nv, nu) pred values out of output[0,0], precomputes the shifted
target-box bounds t_wl/t_ht/t_wr/t_hb per cell (pure functions of
target[0,0] and the offset grid) — and packs one (nv, 8*nu+1) tensor:

    IN = [ x: p_l|p_t|p_r|p_b (4nu) | TB: t_wl|t_ht|t_wr|t_hb (4nu) | T1 ]

with T1 = (t0+t1)*(t2+t3)+1 replicated down the partition axis. The
device evaluates, densely over the nv x nu rectangle (6 DVE ops):

    M2  = min(IN_x, IN_tb)                      # all four mins at once
    C   = [M2_lt|x_lt] + [M2_rb|x_rb]           # [w_int|h_int|lr|tb]
    IP  = C.pairmul                             # [inter | p_area]
    U1  = (p_area + T1) - inter                 # union + 1
    REC = 1 / U1
    RES = (inter + 1) * REC

(M2 is laid out directly before x in one wide SBUF tile so the C step is
a single strided tensor_tensor.) The host writes the returned rectangle
into the zero (W, H) map. Programs are compiled per rectangle shape
(cache key nv, nu); all data-dependent values travel through the input
tensor.

Latency structure (TimelineSim cost model): a DMA costs ~25ns SEQ +
625ns HWDGE + 650ns DGE delay + transfer + 900ns completion-semaphore
propagation, so the kernel is dominated by one input DMA + one output
DMA. _postprocess therefore (a) hoists the input DMA ahead of the
preamble GPR init on SP so it issues at ~25ns, and (b) restructures the
tail so the all-engine barrier runs underneath the output DMA's
completion flight, leaving a single final wait (+ semaphore reset for
re-execution safety) as the last instruction.

Sharding: the op is a single tiny window; all 8 cores run the identical
replicated program (per the sharding hint) and the host reads core 0.
"""

import numpy as np

_TRN_REPO = "/opt/trn_rl_repo"


def _ensure_path():
    import sys

    if _TRN_REPO not in sys.path:
        sys.path.insert(0, _TRN_REPO)


_CACHE = {}
N_CORES = 8


def _build(nv, nu):
    """Bass program: IN (nv, 8*nu+1) -> iou window (nv, nu)."""
    _ensure_path()
    import concourse.bass as bass
    import concourse.tile as tile
    from concourse.tile import add_dep_helper
    from concourse import mybir

    AOT = mybir.AluOpType
    F32 = mybir.dt.float32
    FW = 8 * nu + 1

    nc = bass.Bass("TRN2", debug=False)
    in_d = nc.dram_tensor("x", [nv, FW], F32, kind="ExternalInput").ap()
    iou_d = nc.dram_tensor("iou", [nv, nu], F32, kind="ExternalOutput").ap()

    orders = {"V": []}

    def V(inst):
        orders["V"].append(inst.ins)
        return inst

    with tile.TileContext(nc) as tc:
        with tc.tile_pool(name="sb", bufs=1) as sb:
            # One wide tile: [M2 scratch (4nu) | x (4nu) | tb (4nu) | T1].
            # With M2 laid out directly before x, the pair-sum
            # [M2_lt | x_lt] + [M2_rb | x_rb] is a single strided op whose
            # output C = [w_int | h_int | p_l+p_r | p_t+p_b] feeds one
            # pair-multiply producing [inter | p_area].
            big = sb.tile([nv, 4 * nu + FW], F32)
            xt = big[:, 4 * nu : 4 * nu + FW]
            nc.sync.dma_start(xt, in_d[:])
            x = big[:, 4 * nu : 8 * nu]
            tb = big[:, 8 * nu : 12 * nu]
            t1c = big[:, 12 * nu : 12 * nu + 1]
            m2 = big[:, 0 : 4 * nu]

            V(nc.vector.tensor_tensor(m2, x, tb, AOT.min))
            pq = big[:, 0 : 8 * nu].rearrange("h (i j w) -> h i j w", i=2, j=2)
            C = sb.tile([nv, 4 * nu], F32)
            V(nc.vector.tensor_tensor(
                C[:].rearrange("h (i w) -> h i w", i=2),
                pq[:, :, 0, :], pq[:, :, 1, :], AOT.add))
            C_r = C[:].rearrange("h (i j w) -> h i j w", i=2, j=2)
            IP = sb.tile([nv, 2 * nu], F32)
            V(nc.vector.tensor_tensor(
                IP[:].rearrange("h (i w) -> h i w", i=2),
                C_r[:, :, 0, :], C_r[:, :, 1, :], AOT.mult))
            inter = IP[:, 0:nu]
            pa = IP[:, nu : 2 * nu]
            U1 = sb.tile([nv, nu], F32)
            V(nc.vector.scalar_tensor_tensor(U1[:], pa, t1c, inter,
                                             AOT.add, AOT.subtract))
            REC = sb.tile([nv, nu], F32)
            V(nc.vector.reciprocal(REC[:], U1[:]))
            res = sb.tile([nv, nu], F32)
            V(nc.vector.scalar_tensor_tensor(res[:], inter, 1.0, REC[:],
                                             AOT.add, AOT.mult))

            nc.sync.dma_start(iou_d[:], res[:])
            # Completion semaphore for the output DMA, pinned OUTSIDE the
            # tile semaphore block so the tail's range-clear never touches
            # it: correct in both the timed world (clear runs while the DMA
            # semaphore update is still in flight) and the functional world
            # (update lands immediately). _postprocess points the output
            # DMA's update here and appends the final wait + reset.
            nc.alloc_semaphore("outdone", num=180)

            for seq in orders.values():
                for a, b in zip(seq[1:], seq[:-1]):
                    add_dep_helper(a, b, sync=False, reason="pinned stream order")

    _postprocess(nc)
    return nc


_SPLIT_N = [0]


def _postprocess(nc):
    """BIR surgery, all latency-motivated:

    (0) Hoist the input DMA: SP's preamble GPR inits (zero/bcreg) are not
        read by any SP instruction here, so move them into the tail block;
        SP's first instruction becomes the input DMACopy (~25ns instead of
        ~300ns).
    (1) Tail overlap: the output DMA's completion semaphore takes ~900ns
        to propagate after the transfer. Strip that wait from the tail
        drain so the all-engine barrier runs underneath the flight, and
        append a single final wait on SP, followed by a semaphore reset
        (re-execution safety, since the Pool range-clear runs before the
        DMA semaphore fires).
    (2) This walrus build only supports one sync-wait per instruction;
        hoist extra waits into standalone NoOps on the same engine.
    (3) Drop the dead const-* preamble memsets and the preamble's head
        all-engine barrier (cross-engine deps are all carried by tile
        semaphores; the tail barrier is what guards re-execution)."""
    _ensure_path()
    from concourse import mybir

    ET = mybir.EngineType

    fns = list(nc.m.functions)
    blocks = {b.name: b for f in fns for b in f.blocks}
    main = blocks.get("main")
    build = end = None
    for name, b in blocks.items():
        if name.endswith("__build") or (name != "main" and not name.endswith("_end")
                                        and build is None):
            build = b
        if name.endswith("_end"):
            end = b

    # --- (0) hoist SP preamble GPR inits into the tail block, and pull the
    # input DMACopy into the main block ahead of SP's branch, so the input
    # DMA is the very first SP instruction (~25ns instead of ~300ns) ---
    if main is not None and end is not None:
        sp_regmoves = [i for i in main.instructions
                       if isinstance(i, mybir.InstRegisterMove)
                       and i.engine == ET.SP]
        if sp_regmoves:
            main.instructions = [i for i in main.instructions
                                 if i not in sp_regmoves]
            end.instructions = sp_regmoves + list(end.instructions)
    if main is not None and build is not None:
        in_dma = next((i for i in build.instructions
                       if isinstance(i, mybir.InstDMACopy)
                       and i.engine == ET.SP
                       and not (i.sync_info and i.sync_info.on_wait)), None)
        if in_dma is not None:
            build.instructions = [i for i in build.instructions
                                  if i is not in_dma]
            mains = list(main.instructions)
            ix = next((k for k, i in enumerate(mains)
                       if isinstance(i, mybir.InstUnconditionalBranch)
                       and i.engine == ET.SP), len(mains))
            main.instructions = mains[:ix] + [in_dma] + mains[ix:]

    # --- (1) tail overlap for the output DMA semaphore ---
    # Repoint the output DMA's completion update at the pinned "outdone"
    # semaphore (index 180, outside the tile block the tail range-clear
    # wipes), strip the tail's wait on the old tile-lane semaphore so the
    # all-engine barrier runs underneath the DMA flight, and end the
    # program with wait(outdone>=16) + reset on SP (separate instructions:
    # walrus rejects wait+update of one semaphore on one instruction).
    out_dma = None
    if build is not None:
        for inst in build.instructions:
            if isinstance(inst, mybir.InstDMACopy):
                si = inst.sync_info
                if si and si.on_update:
                    out_dma = inst  # last DMACopy wins
    if out_dma is not None and end is not None:
        old_upd = out_dma.sync_info.on_update[0]
        out_dma.sync_info.on_update = [mybir.SyncUpdate(
            sync_type="semaphore", id=180, ant_name="outdone",
            update_mode="sem-add-imm", update_value=16, update_reg=None)]
        for inst in end.instructions:
            si = inst.sync_info
            if si is None or not si.on_wait:
                continue
            kept = [w for w in si.on_wait if w.ant_name != old_upd.ant_name]
            if len(kept) != len(si.on_wait):
                si.on_wait = kept
        final_wait = mybir.InstNoOp(name="final-dma-wait")
        final_wait.engine = ET.SP
        final_wait.sync_info = mybir.SyncInfo(
            on_wait=[mybir.SyncWait(
                sync_type="semaphore", id=180, ant_name="outdone",
                wait_mode="sem-ge-imm", wait_value=16, wait_reg=None)],
            on_update=[])
        final_clear = mybir.InstNoOp(name="final-dma-sem-clear")
        final_clear.engine = ET.SP
        final_clear.sync_info = mybir.SyncInfo(
            on_wait=[],
            on_update=[mybir.SyncUpdate(
                sync_type="semaphore", id=180, ant_name="outdone",
                update_mode="sem-sub-imm", update_value=16, update_reg=None)])
        end.instructions = list(end.instructions) + [final_wait, final_clear]

    # NOTE: same-engine DVE->DVE semaphore waits look redundant (engines
    # issue in order; the DVE pipe flushes between ops per the microarch
    # doc) but removing them breaks execution on this toolchain — the
    # compiler/runtime relies on the semaphore edges for instruction
    # ordering. They stay.

    # --- (2) + (3) ---
    for f in fns:
        for b in f.blocks:
            insts = b.instructions
            new = []
            changed = False
            for inst in insts:
                if b.name == "main" and isinstance(
                    inst, mybir.InstDrain | mybir.InstEventSemaphore
                ):
                    changed = True
                    continue
                if (
                    isinstance(inst, mybir.InstMemset)
                    and inst.outs
                    and getattr(inst.outs[0], "memref", "").startswith("const-")
                    and not (inst.sync_info and (inst.sync_info.on_wait
                                                 or inst.sync_info.on_update))
                ):
                    changed = True
                    continue
                si = inst.sync_info
                if si is not None and si.on_wait and len(si.on_wait) > 1:
                    waits = list(si.on_wait)
                    for w in waits[:-1]:
                        _SPLIT_N[0] += 1
                        n = mybir.InstNoOp(name=f"splitwait-{_SPLIT_N[0]}")
                        n.engine = inst.engine
                        n.sync_info = mybir.SyncInfo(on_wait=[w], on_update=[])
                        new.append(n)
                    si.on_wait = waits[-1:]
                    changed = True
                new.append(inst)
            if changed:
                b.instructions = new


def _get_program(nv, nu):
    key = (nv, nu)
    if key not in _CACHE:
        _CACHE[key] = _build(nv, nu)
    return _CACHE[key]


def _pack_inputs(output, ind, target, radius):
    """Host-side window extraction + constant precompute.

    All three validity conditions (shifted target box nonnegative, window
    offset within radius, center+offset inside the image) are intervals in
    the row/column offsets, so the valid cells form an exact rectangle
    [v_lo..v_hi] x [u_lo..u_hi] around the center. Only that rectangle is
    shipped to the device — no padding, no mask.

    Returns (W, vh, wl, xin) where (vh, wl) is the top-left corner of the
    rectangle in the full map and xin the (nv, 8*nu+1) device input, or
    xin=None when the rectangle is empty."""
    output = np.asarray(output)
    W, H = output.shape[-2], output.shape[-1]
    assert W == H
    dim = 4
    R = int(radius)
    out0 = np.asarray(output, dtype=np.float32).reshape(-1, dim, W, H)[0]
    tgt = np.asarray(target, dtype=np.float32).reshape(-1, dim)[0]
    t0, t1, t2, t3 = (float(v) for v in tgt)
    ind0 = int(np.asarray(ind).reshape(-1)[0])
    ch, cw = ind0 // W, ind0 % W

    v_lo = max(int(np.ceil(-t2)), -ch, -R)
    v_hi = min(int(np.floor(t3)), W - 1 - ch, R)
    u_lo = max(int(np.ceil(-t0)), -cw, -R)
    u_hi = min(int(np.floor(t1)), W - 1 - cw, R)
    if v_lo > v_hi or u_lo > u_hi:
        return W, 0, 0, None

    nv, nu = v_hi - v_lo + 1, u_hi - u_lo + 1
    sub = out0[:, ch + v_lo : ch + v_hi + 1, cw + u_lo : cw + u_hi + 1]
    x4 = np.ascontiguousarray(sub.transpose(1, 0, 2))[:, [0, 2, 1, 3], :]
    # channel blocks [p_l | p_t | p_r | p_b]

    uf = np.arange(u_lo, u_hi + 1, dtype=np.float32)
    vf = np.arange(v_lo, v_hi + 1, dtype=np.float32)
    tb4 = np.empty((nv, dim, nu), dtype=np.float32)
    tb4[:, 0, :] = t0 + uf[None, :]  # t_wl(u)
    tb4[:, 1, :] = (t2 + vf)[:, None]  # t_ht(v)
    tb4[:, 2, :] = t1 - uf[None, :]  # t_wr(u)
    tb4[:, 3, :] = (t3 - vf)[:, None]  # t_hb(v)

    xin = np.empty((nv, 8 * nu + 1), dtype=np.float32)
    xin[:, 0 : 4 * nu] = x4.reshape(nv, 4 * nu)
    xin[:, 4 * nu : 8 * nu] = tb4.reshape(nv, 4 * nu)
    xin[:, 8 * nu] = (t0 + t1) * (t2 + t3) + 1.0
    return W, ch + v_lo, cw + u_lo, xin


def kernel(output, ind, target, radius):
    _ensure_path()
    from concourse.bass_utils import run_bass_kernel_spmd

    W, vh, wl, xin = _pack_inputs(output, ind, target, radius)
    iou_map = np.zeros((W, W), dtype=np.float32)
    if xin is None:
        return iou_map
    nv, nu = xin.shape[0], (xin.shape[1] - 1) // 8
    nc = _get_program(nv, nu)
    res = run_bass_kernel_spmd(nc, [{"x": xin} for _ in range(N_CORES)],
                               core_ids=list(range(N_CORES)))
    iou_map[vh : vh + nv, wl : wl + nu] = np.asarray(res.results[0]["iou"])
    return iou_map
